# revision 4
# baseline (speedup 1.0000x reference)
"""Causal single-head attention on 8 TRN2 NeuronCores, data-parallel over batch.

Per-core problem (batch element b = core id):
  x [2048, 768] f32, Wq/Wk/Wv [768, 64] f32 -> out [2048, 64] f32
  out = softmax(causal((x Wq)(x Wk)^T / 8)) @ (x Wv)

Design notes:
- All matmul operands in bf16 (f32 PSUM accumulation); measured end-to-end
  rel err ~4e-3 vs f32 reference.
- Scores are computed TRANSPOSED (S^T[k, q]) so that the exp'd tile P~^T can
  feed the PV matmul directly as the stationary operand (contraction over k
  on partitions) without transposing the 2048x2048 attention matrix.
- No row-max subtraction: scores ~ N(0,1), |S|max ~ 7, exp is safe in f32.
- Softmax denominator comes free from a ones-column appended to V: the PV
  matmul accumulates [O^T; rowsum] in one PSUM tile (M=65).
- x^T tiles (d on partitions) produced by f32->bf16 cast-DMA (SWDGE) plus
  xbar transpose DMAs (dma_start_transpose, out[p,c,f] = in[f, 128c+p]).
"""

import numpy as np

import concourse.bass as bass
import concourse.mybir as mybir
import concourse.tile as tile
from concourse import bacc
from concourse.bass import ts
from concourse.bass_utils import run_bass_kernel_spmd
from concourse.masks import make_identity, make_upper_triangular

F32 = mybir.dt.float32
BF16 = mybir.dt.bfloat16
AF = mybir.ActivationFunctionType

B, T, D, H = 8, 2048, 768, 64
NT = T // 128   # 16 t-blocks
ND = D // 128   # 6 d-chunks
NKB = T // 128  # 16 key blocks
NQS = T // 512  # 4 query supers
QS = 512


def build():
    nc = bacc.Bacc("TRN2", target_bir_lowering=False, debug=False)
    x_ext = nc.dram_tensor("x", [T, D], F32, kind="ExternalInput").ap()
    wq_ext = nc.dram_tensor("Wq", [D, H], F32, kind="ExternalInput").ap()
    wk_ext = nc.dram_tensor("Wk", [D, H], F32, kind="ExternalInput").ap()
    wv_ext = nc.dram_tensor("Wv", [D, H], F32, kind="ExternalInput").ap()
    out_ext = nc.dram_tensor("out", [T, H], F32, kind="ExternalOutput").ap()

    with tile.TileContext(nc) as tc:
        with (
            tc.tile_pool(name="const", bufs=1) as constp,
            tc.tile_pool(name="xp", bufs=1) as xp,
            tc.tile_pool(name="wp", bufs=1) as wp,
            tc.tile_pool(name="qk", bufs=1) as qkp,
            tc.tile_pool(name="pt", bufs=1) as ptp,
            tc.tile_pool(name="fin", bufs=2) as finp,
            tc.tile_pool(name="psS", bufs=3, space="PSUM") as psSp,
            tc.tile_pool(name="psT", bufs=2, space="PSUM") as psTp,
            tc.tile_pool(name="psO", bufs=2, space="PSUM") as psOp,
        ):
            # ---- constants ----
            ident = constp.tile([128, 128], F32)
            make_identity(nc, ident[:])
            tri = constp.tile([128, 128], BF16)
            # tri[k, q] = 1 if q >= k else 0  (keep q>=k in S^T diag blocks)
            make_upper_triangular(nc, tri[:], val=1.0, diag=True)

            # ---- load x (cast f32->bf16) and weights ----
            x_bf = xp.tile([128, NT, D], BF16)
            for tb in range(NT):
                nc.gpsimd.dma_start(out=x_bf[:, tb], in_=x_ext[ts(tb, 128), :])
            w_bf = wp.tile([128, 3, ND, H], BF16)
            for wi, w_ext in enumerate((wq_ext, wk_ext, wv_ext)):
                nc.gpsimd.dma_start(
                    out=w_bf[:, wi],
                    in_=w_ext.rearrange("(c p) h -> p c h", p=128),
                )

            # ---- x^T via xbar transpose: xT[p, c, t] = x[t, 128c+p] ----
            xT_bf = xp.tile([128, ND, T], BF16)
            for tb in range(NT):
                nc.sync.dma_start_transpose(
                    out=xT_bf[:, :, ts(tb, 128)], in_=x_bf[:, tb]
                )

            # ---- QKV projections: Q^T/K^T/V^T [64, T] (h on partitions) ----
            qT = qkp.tile([64, T], BF16)
            kT = qkp.tile([64, T], BF16)
            vT = qkp.tile([64, T], BF16)
            for tsb in range(NQS):
                for wi, dst in enumerate((qT, kT, vT)):
                    ps = psSp.tile([64, QS], F32, tag="psS", name="psqkv")
                    for dc in range(ND):
                        nc.tensor.matmul(
                            ps[:],
                            w_bf[:, wi, dc],
                            xT_bf[:, dc, ts(tsb, QS)],
                            start=(dc == 0),
                            stop=(dc == ND - 1),
                        )
                    nc.vector.tensor_copy(dst[:, ts(tsb, QS)], ps[:])

            # ---- V3[p, kb, 0:64] = V[t=128kb+p, h]; V3[:, :, 64] = 1.0 ----
            v3c = finp.tile([128, NKB, H], BF16, tag="v3c")
            nc.sync.dma_start_transpose(out=v3c[:], in_=vT[:])
            v3 = qkp.tile([128, NKB, H + 1], BF16)
            nc.vector.tensor_copy(v3[:, :, 0:H], v3c[:])
            nc.vector.memset(v3[:, :, H : H + 1], 1.0)

            # ---- scores^T + exp, per key block ----
            # pT[kb][k, j] = exp(S[q, k] / 8) for q = 128*kb + j  (j in [0, w))
            pT = []
            for kb in range(NKB):
                w = T - 128 * kb
                pt_tile = ptp.tile([128, w], BF16, tag=f"pt{kb}", name=f"pt{kb}")
                pT.append(pt_tile)
            for kb in range(NKB):
                w = T - 128 * kb
                q0 = 128 * kb
                for c in range(0, w, QS):
                    n = min(QS, w - c)
                    ps = psSp.tile([128, QS], F32, tag="psS")
                    nc.tensor.matmul(
                        ps[:, 0:n],
                        kT[:, ts(kb, 128)],
                        qT[:, q0 + c : q0 + c + n],
                        start=True,
                        stop=True,
                    )
                    nc.scalar.activation(
                        pT[kb][:, c : c + n], ps[:, 0:n], AF.Exp, scale=0.125
                    )
                # causal mask on the diagonal 128 columns (q_local 0..127)
                nc.vector.tensor_mul(pT[kb][:, 0:128], pT[kb][:, 0:128], tri[:])

            # ---- PV: accumulate [O^T; rowsum] per query super, then finalize ----
            for qs in range(NQS):
                pso = psOp.tile([H + 1, QS], F32, tag="psO")
                last_kb = 4 * qs + 3
                for kb in range(last_kb + 1):
                    qlo = QS * qs - 128 * kb  # offset of this q-super in pT[kb]
                    off = max(0, -qlo)
                    nc.tensor.matmul(
                        pso[:, off:QS],
                        v3[:, kb],
                        pT[kb][:, max(0, qlo) : qlo + QS],
                        start=(kb == 0),
                        stop=(kb == last_kb),
                    )
                of = finp.tile([H + 1, QS], F32, tag="of")
                nc.vector.tensor_copy(of[:], pso[:])
                for j in range(QS // 128):
                    pst = psTp.tile([128, H + 1], F32, tag="psT")
                    nc.tensor.transpose(
                        pst[:], of[:, ts(j, 128)], ident[0 : H + 1, 0 : H + 1]
                    )
                    rec = finp.tile([128, 1], F32, tag="rec")
                    nc.vector.reciprocal(rec[:], pst[:, H : H + 1])
                    ob = finp.tile([128, H], F32, tag="ob")
                    nc.vector.tensor_scalar_mul(ob[:], pst[:, 0:H], rec[:])
                    nc.gpsimd.dma_start(
                        out=out_ext[ts(4 * qs + j, 128), :], in_=ob[:]
                    )

    nc.compile()
    return nc


_nc_cache = None


def _get_nc():
    global _nc_cache
    if _nc_cache is None:
        _nc_cache = build()
    return _nc_cache


def kernel(x, Wq, Wk, Wv, _trace=False):
    nc = _get_nc()
    x = np.ascontiguousarray(np.asarray(x, dtype=np.float32))
    Wq = np.ascontiguousarray(np.asarray(Wq, dtype=np.float32))
    Wk = np.ascontiguousarray(np.asarray(Wk, dtype=np.float32))
    Wv = np.ascontiguousarray(np.asarray(Wv, dtype=np.float32))
    in_maps = [
        {"x": x[i], "Wq": Wq, "Wk": Wk, "Wv": Wv} for i in range(B)
    ]
    res = run_bass_kernel_spmd(nc, in_maps, core_ids=list(range(B)), trace=_trace)
    out = np.stack([res.results[i]["out"] for i in range(B)])
    if _trace:
        kernel.last_exec_time_ns = res.exec_time_ns
        kernel.last_results = res
    return out
